# revision 23
# baseline (speedup 1.0000x reference)
"""ChaosAttention on 8 Trainium2 NeuronCores.

Sharding: tensor-parallel over heads. Each of the 8 cores owns H/8 = 2 heads
(128 of the 1024 q/k/v projection columns, 128 of the Wo rows). Every core
reads the full x (as x^T, bf16); the out-projection is row-parallel, so each
core returns a partial y^T (bf16) and the host sums the 8 partials (f64) and
adds bo.

The physics adapter produces a bias constant along the softmax axis, so it
cancels in the softmax and is skipped entirely.

Key compaction: masked keys get -inf scores in the reference, so the host
packs only the kept keys per batch (padded to a multiple of 128; pad slots
get a -1e30 exp bias), halving attention work for a ~50% mask.

v4 notes (vs the 187us baseline):
  - fp8 was tried and reverted: the attention output is an average over
    ~780 keys, so o is ~28x smaller than v and every quantization error
    keeps its RELATIVE size through the averaging - fp8 anywhere in the
    q/k/v path gives 1-3% output error vs the 2% budget.
  - v-projection computed transposed (8 wide matmuls like k) then rotated
    back with 4 PE transposes per window: a PE matmul carries ~170ns fixed
    overhead, so the old 32x 128-column form burned ~20us of PE across the
    kernel. The transposes write bf16 into a bitcast view of a PSUM f32
    tile (PSUM banks are all spoken for).
  - yT returned bf16 (host sums partials in f64): halves output DMA.
  - 1/Z via vector.reciprocal_approx_fast (~5x cheaper than reciprocal);
    custom-DVE ops and Pool broadcasts need partition-0-based operands.
  - normalize mul rides the Pool engine (all-SBUF); out-proj PSUM->SBUF
    copies split 7:1 DVE:ACT (Pool cannot touch PSUM).
  - Each HWDGE engine (SP/ACT only) owns ONE FIFO dma queue and a transfer
    carries ~0.7us fixed overhead: traffic is balanced across both queues
    in consumption order with ~512KB transfers. ACT only carries what must
    land before its exp stream starts; tiny loads ride the Pool SWDGE
    queue. y writes are quad-merged.
  - The first attention chunk is gated on [k00,q00,v00]; everything else
    interleaves into the chunk jt-loops (2 pops/jt) including the previous
    chunk's out-proj units at jt>=5, which keeps the final-chunk drain
    short.
"""

import numpy as np
import ml_dtypes

_BF16 = ml_dtypes.bfloat16

B, T, E, H, D = 2, 2048, 1024, 16, 64
BT = B * T                 # 4096 tokens
N_CORES = 8
PCN = E // N_CORES         # 128 per-core projection dims (2 heads x 64)
EC = E // 128              # 8 contraction chunks for the projections
IC = 512                   # query-chunk size
NQC = T // IC              # 4 query chunks per batch
SCALE = 1.0 / float(np.sqrt(D))

_cache = {}


def _build(nkt, zero_bv):
    """Build + schedule the per-core Bass program. nkt = packed key-tile
    counts per batch (ceil(kept/128))."""
    key = (tuple(nkt), zero_bv)
    if key in _cache:
        return _cache[key]

    from contextlib import ExitStack
    import concourse.tile as tile
    from concourse import bacc, mybir

    f32 = mybir.dt.float32
    bf16 = mybir.dt.bfloat16
    Exp = mybir.ActivationFunctionType.Exp

    base = [0, nkt[0]]              # packed key-tile offset per batch
    ntt = nkt[0] + nkt[1]           # total packed key tiles
    KP = ntt * 128                  # total packed+padded kv tokens

    nc = bacc.Bacc("TRN2", target_bir_lowering=False, debug=False,
                   num_devices=N_CORES)

    xT_d = nc.dram_tensor("xT", [128, EC, BT], bf16, kind="ExternalInput").ap()
    xk_d = nc.dram_tensor("xk", [128, EC, KP], bf16, kind="ExternalInput").ap()
    wq_d = nc.dram_tensor("wq", [128, EC, PCN], bf16, kind="ExternalInput").ap()
    wk_d = nc.dram_tensor("wk", [128, EC, PCN], bf16, kind="ExternalInput").ap()
    wv_d = nc.dram_tensor("wv", [128, EC, PCN], bf16, kind="ExternalInput").ap()
    wo_d = nc.dram_tensor("wo", [PCN, E], bf16, kind="ExternalInput").ap()
    id_d = nc.dram_tensor("ident", [128, 128], bf16, kind="ExternalInput").ap()
    bq_d = nc.dram_tensor("bq", [PCN, 1], f32, kind="ExternalInput").ap()
    bk_d = nc.dram_tensor("bk", [PCN, 1], f32, kind="ExternalInput").ap()
    bv_d = nc.dram_tensor("bv", [PCN, 1], f32, kind="ExternalInput").ap()
    mb_d = nc.dram_tensor("mb", [128, ntt], f32, kind="ExternalInput").ap()
    yT_d = nc.dram_tensor("yT", [E, BT], bf16, kind="ExternalOutput").ap()

    def kwins(b):
        """(start, size) 512-col windows over batch b's packed kv columns."""
        n = nkt[b] * 128
        return [(w * 512, min(512, n - w * 512)) for w in range((n + 511) // 512)]

    with tile.TileContext(nc) as tc, ExitStack() as ctx:
        consts = ctx.enter_context(tc.tile_pool(name="consts", bufs=1))
        pp_mm = ctx.enter_context(tc.tile_pool(name="ppmm", bufs=2, space="PSUM"))
        pp_st = ctx.enter_context(tc.tile_pool(name="ppst", bufs=2, space="PSUM"))
        pp_o = ctx.enter_context(tc.tile_pool(name="ppo", bufs=2, space="PSUM"))
        pool_pt = ctx.enter_context(tc.tile_pool(name="ptp", bufs=3))
        pool_vt = ctx.enter_context(tc.tile_pool(name="vtp", bufs=2))
        pool_oc = ctx.enter_context(tc.tile_pool(name="ocp", bufs=2))
        pool_rz = ctx.enter_context(tc.tile_pool(name="rzp", bufs=2))
        pool_rb = ctx.enter_context(tc.tile_pool(name="rbp", bufs=2))
        pool_y = ctx.enter_context(tc.tile_pool(name="yp", bufs=4))

        # ---- persistent SBUF residents ----
        xq = [consts.tile([128, EC, 512], bf16, tag=f"xq{mw}", name=f"xq{mw}")
              for mw in range(8)]
        xkw = {}
        for b in range(B):
            for wi, (w0, wsz) in enumerate(kwins(b)):
                xkw[(b, wi)] = consts.tile([128, EC, wsz], bf16,
                                           tag=f"xk{b}_{wi}", name=f"xk{b}_{wi}")

        wq_sb = consts.tile([128, EC, PCN], bf16, tag="wq")
        wk_sb = consts.tile([128, EC, PCN], bf16, tag="wk")
        wv_sb = consts.tile([128, EC, PCN], bf16, tag="wv")
        wo_sb = consts.tile([128, E], bf16, tag="wo")
        id_sb = consts.tile([128, 128], bf16, tag="ident")
        mb_sb = consts.tile([128, ntt], f32, tag="mb")
        bq_sb = consts.tile([128, 1], f32, tag="bq")
        bk_sb = consts.tile([128, 1], f32, tag="bk")
        bv_sb = consts.tile([128, 1], f32, tag="bv")

        def dma_xk(eng, b, wi, splits):
            w0, wsz = kwins(b)[wi]
            c0 = base[b] * 128 + w0
            e0 = 0
            for step in splits:
                nc_eng = eng
                nc_eng.dma_start(xkw[(b, wi)][:, e0:e0 + step, :],
                                 xk_d[:, e0:e0 + step, c0:c0 + wsz])
                e0 += step

        def dma_xq(eng, mw, splits):
            e0 = 0
            for step in splits:
                eng.dma_start(xq[mw][:, e0:e0 + step, :],
                              xT_d[:, e0:e0 + step, mw * 512:(mw + 1) * 512])
                e0 += step

        # DMA layout: ACT carries only what must land before its exp stream
        # starts; SP carries the rest of the inputs + mid-stream y writes;
        # tiny loads ride the Pool SWDGE queue.
        dma_xk(nc.sync, 0, 0, (2, 2, 4))
        for w_sb, w_d in ((wk_sb, wk_d), (wq_sb, wq_d), (wv_sb, wv_d)):
            nc.scalar.dma_start(w_sb[:], w_d[:])
        nc.gpsimd.dma_start(id_sb[:], id_d[:])
        nc.gpsimd.dma_start(mb_sb[:], mb_d[:])
        nc.gpsimd.dma_start(bq_sb[:], bq_d[:])
        nc.gpsimd.dma_start(bk_sb[:], bk_d[:])
        if not zero_bv:
            nc.gpsimd.dma_start(bv_sb[:], bv_d[:])
        dma_xq(nc.scalar, 0, (2, 2, 4))
        for wi in range(1, len(kwins(0))):
            dma_xk(nc.sync, 0, wi, (4, 4))
        dma_xq(nc.sync, 1, (4, 4))
        dma_xq(nc.sync, 2, (4, 4))
        for wi in range(len(kwins(1))):
            dma_xk(nc.sync, 1, wi, (4, 4))
        dma_xq(nc.sync, 3, (4, 4))
        nc.sync.dma_start(wo_sb[:], wo_d[:])
        for mw in range(4, 8):
            dma_xq(nc.sync, mw, (4, 4))

        # per-chunk tiles: fine-grained deps let attention start early
        qTc = {(b, icx): consts.tile([128, IC], bf16, tag=f"qT{b}_{icx}",
                                     name=f"qT{b}_{icx}")
               for b in range(B) for icx in range(NQC)}
        ATc = {(b, icx): consts.tile([128, IC], bf16, tag=f"AT{b}_{icx}",
                                     name=f"AT{b}_{icx}")
               for b in range(B) for icx in range(NQC)}
        kTc = {}
        Vpg = {}
        for b in range(B):
            for wi, (w0, wsz) in enumerate(kwins(b)):
                kTc[(b, wi)] = consts.tile([128, wsz], bf16, tag=f"kT{b}_{wi}",
                                           name=f"kT{b}_{wi}")
                Vpg[(b, wi)] = consts.tile([128, wsz // 128, 2, D + 1], bf16,
                                           tag=f"Vp{b}_{wi}", name=f"Vp{b}_{wi}")
                nc.gpsimd.memset(Vpg[(b, wi)][:, :, :, D:D + 1], 1.0)

        # ---- phase emitters ----
        def proj_q_chunk(b, icx):
            mw = (b * T + icx * IC) // 512
            ps = pp_mm.tile([128, 512], f32, tag="mm", name="psq")
            for ec in range(EC):
                nc.tensor.matmul(ps[:], lhsT=wq_sb[:, ec, :],
                                 rhs=xq[mw][:, ec, :],
                                 start=(ec == 0), stop=(ec == EC - 1))
            nc.vector.tensor_scalar_add(out=qTc[(b, icx)][:], in0=ps[:],
                                        scalar1=bq_sb[:])

        def proj_k_chunk(b, wi, wsz):
            ps = pp_mm.tile([128, 512], f32, tag="mm", name="psk")
            for ec in range(EC):
                nc.tensor.matmul(ps[:, 0:wsz], lhsT=wk_sb[:, ec, :],
                                 rhs=xkw[(b, wi)][:, ec, :],
                                 start=(ec == 0), stop=(ec == EC - 1))
            nc.vector.tensor_scalar_add(out=kTc[(b, wi)][:], in0=ps[:, 0:wsz],
                                        scalar1=bk_sb[:])

        vt_st = {}

        def proj_v_mm(b, wi, wsz):
            # v computed transposed (vdims on partitions): 8 wide matmuls
            # instead of 32 overhead-bound 128-column ones
            ps = pp_mm.tile([128, 512], f32, tag="mm", name="psv")
            for ec in range(EC):
                nc.tensor.matmul(ps[:, 0:wsz], lhsT=wv_sb[:, ec, :],
                                 rhs=xkw[(b, wi)][:, ec, :],
                                 start=(ec == 0), stop=(ec == EC - 1))
            vt = pool_vt.tile([128, 512], bf16, tag="vt", name="vt")
            if zero_bv:
                nc.vector.tensor_copy(out=vt[:, 0:wsz], in_=ps[:, 0:wsz])
            else:
                nc.vector.tensor_scalar_add(out=vt[:, 0:wsz], in0=ps[:, 0:wsz],
                                            scalar1=bv_sb[:])
            vt_st[(b, wi)] = vt

        def proj_v_tp(b, wi, wsz):
            # rotate v back to natural layout (tokens on partitions) with PE
            # transposes, staging bf16 through a bitcast view of an f32 PSUM
            # tile; then pack into Vpg next to its ones-column
            nt = wsz // 128
            vt = vt_st.pop((b, wi))
            tp = pp_mm.tile([128, 512], f32, tag="mm", name="pst")
            tpb = tp[:].bitcast(bf16)
            for q in range(nt):
                nc.tensor.transpose(tpb[:, q * 128:(q + 1) * 128],
                                    vt[:, q * 128:(q + 1) * 128], id_sb[:])
            nc.vector.tensor_copy(
                out=Vpg[(b, wi)][:, :, :, 0:D],
                in_=tpb[:, 0:wsz].rearrange("p (q h d) -> p q h d", q=nt, h=2))

        def proj_items(b):
            items = []
            wins = kwins(b)
            for wi, (w0, wsz) in enumerate(wins):
                items.append(lambda b=b, wi=wi, wsz=wsz: proj_k_chunk(b, wi, wsz))
                items.append(lambda b=b, wi=wi, wsz=wsz: proj_v_mm(b, wi, wsz))
                items.append(lambda b=b, wi=wi, wsz=wsz: proj_v_tp(b, wi, wsz))
            items.insert(1, lambda b=b: proj_q_chunk(b, 0))
            for icx in range(1, NQC):
                items.append(lambda b=b, icx=icx: proj_q_chunk(b, icx))
            return items

        def outproj_unit(b, icx, et, last=False):
            yp = pp_mm.tile([128, 512], f32, tag="mm", name="psy")
            nc.tensor.matmul(yp[:], lhsT=wo_sb[:, et * 128:(et + 1) * 128],
                             rhs=ATc[(b, icx)][:], start=True, stop=True)
            ysb = pool_y.tile([128, 4, 512], bf16, tag="y",
                              name=f"y{et // 4}") if et % 4 == 0 else \
                outproj_unit.cur
            outproj_unit.cur = ysb
            # PSUM is only reachable from PE/ACT/DVE; one copy per chunk
            # rides ACT, the rest DVE
            if et == 3:
                nc.scalar.copy(out=ysb[:, et % 4, :], in_=yp[:])
            else:
                nc.vector.tensor_copy(out=ysb[:, et % 4, :], in_=yp[:])
            if et % 4 == 3:
                i0 = b * T + icx * IC
                dst = yT_d[(et - 3) * 128:(et + 1) * 128, i0:i0 + 512]
                # the final chunk drains into the kernel tail on the ACT
                # queue, which is idle once the exps are done
                eng = nc.scalar if last else nc.sync
                eng.dma_start(dst.rearrange("(j p) n -> p j n", p=128), ysb[:])

        def outproj_units(b, icx):
            last = (b == B - 1 and icx == NQC - 1)
            return [lambda et=et: outproj_unit(b, icx, et, last)
                    for et in range(EC)]

        def attn_chunk(b, icx, light, late, heavy, norm_prev):
            o_ps = [pp_o.tile([D + 1, IC], f32, tag="o", name=f"o{h}")
                    for h in range(2)]
            for jt in range(nkt[b]):
                tg = base[b] + jt
                wi, q = jt // 4, jt % 4
                st = pp_st.tile([128, 2 * IC], f32, tag="st")
                nc.tensor.matmul(st[:, 0:IC],
                                 lhsT=kTc[(b, wi)][0:64, q * 128:(q + 1) * 128],
                                 rhs=qTc[(b, icx)][0:64, :],
                                 tile_position=(0, 0), start=True, stop=True)
                nc.tensor.matmul(st[:, IC:2 * IC],
                                 lhsT=kTc[(b, wi)][64:128, q * 128:(q + 1) * 128],
                                 rhs=qTc[(b, icx)][64:128, :],
                                 tile_position=(64, 0), start=True, stop=True)
                pt = pool_pt.tile([128, 2 * IC], bf16, tag="pt")
                nc.scalar.activation(out=pt[:], in_=st[:], func=Exp,
                                     bias=mb_sb[:, tg:tg + 1], scale=SCALE)
                for h in range(2):
                    nc.tensor.matmul(o_ps[h][:], lhsT=Vpg[(b, wi)][:, q, h, :],
                                     rhs=pt[:, h * IC:(h + 1) * IC],
                                     start=(jt == 0), stop=(jt == nkt[b] - 1))
                for _ in range(2):
                    if light:
                        light.pop(0)()
                if norm_prev and jt >= 1:
                    for _ in range(2):
                        if norm_prev:
                            norm_prev.pop(0)()
                if jt >= 5:
                    for _ in range(2):
                        if late:
                            late.pop(0)()
                if jt % 3 == 1 and heavy:
                    heavy.pop(0)()
            # free the o_ps slots fast: only the PSUM->SBUF copies happen at
            # the boundary; the reciprocal/broadcast/mul tail is deferred
            # into the next chunk, keeping ACT on back-to-back exps.
            ocs = []
            for h in range(2):
                oc = pool_oc.tile([D + 1, IC], f32, tag="oc", name=f"oc{h}")
                nc.vector.tensor_copy(out=oc[:], in_=o_ps[h][:])
                ocs.append(oc)

            st8 = {}

            def p_recip(h):
                # custom-DVE reciprocal and Pool broadcast both require
                # partition-0-based operands: gather Z to partition 0 first
                zz = pool_rz.tile([1, IC], f32, tag=f"zz{h}", name=f"zz{h}")
                nc.vector.tensor_copy(out=zz[:], in_=ocs[h][D:D + 1, :])
                rz = pool_rz.tile([1, IC], f32, tag=f"rz{h}", name=f"rz{h}")
                nc.vector.reciprocal_approx_fast(out=rz[:], in_=zz[:])
                st8[("rz", h)] = rz

            def p_bcast(h):
                rb = pool_rb.tile([D, IC], f32, tag=f"rb{h}", name=f"rb{h}")
                nc.gpsimd.partition_broadcast(rb[:], st8[("rz", h)][0:1, :])
                st8[("rb", h)] = rb

            def p_mul(h):
                # all-SBUF operands: ride the idle Pool engine
                at = ATc[(b, icx)][D * h:D * (h + 1), :]
                nc.gpsimd.tensor_mul(out=at, in0=ocs[h][0:D, :],
                                     in1=st8[("rb", h)][:])

            return [lambda: p_recip(0), lambda: p_recip(1),
                    lambda: p_bcast(0), lambda: p_bcast(1),
                    lambda: p_mul(0), lambda: p_mul(1)]

        # ---- program ----
        # only the pieces chunk (0,0) needs before its jt loop go upfront;
        # the rest of b0's projections interleave into the chunk jt-loops
        items0 = proj_items(0)
        for it in items0[:4]:
            it()
        ready = items0[4:]
        heavy = proj_items(1)
        delay, norm2 = [], None
        for b in range(B):
            for icx in range(NQC):
                norm2 = attn_chunk(b, icx, ready, delay, heavy, norm2)
                ready.extend(delay)
                delay = outproj_units(b, icx)
        while heavy:
            heavy.pop(0)()
        for p in norm2:
            p()
        while ready:
            ready.pop(0)()
        for u in delay:
            u()

    nc.compile()
    _cache[key] = nc
    return nc


def _prepare(x, attn_mask, Wq, bq, Wk, bk, Wv, bv, Wo):
    mask = np.asarray(attn_mask).astype(bool)
    xf = np.asarray(x, dtype=np.float32).reshape(B, T, E)

    nkt = []
    cols = []       # packed kv token features, (KP, E) f32
    mbcols = []     # per packed slot: 0 keep / -1e30 pad
    for b in range(B):
        idx = np.nonzero(mask[b])[0]
        nk = len(idx)
        ntiles = max(1, (nk + 127) // 128)
        npad = ntiles * 128
        feats = np.zeros((npad, E), dtype=np.float32)
        feats[:nk] = xf[b, idx, :]
        bias = np.full(npad, -1e30, dtype=np.float32)
        bias[:nk] = 0.0
        nkt.append(ntiles)
        cols.append(feats)
        mbcols.append(bias)

    def chunked_T(a):
        # (N, E) f32 -> [128, EC, N] bf16 where [p, c, n] = a[n, c*128+p]
        return np.ascontiguousarray(
            a.T.reshape(EC, 128, -1).transpose(1, 0, 2)).astype(_BF16)

    xk = chunked_T(np.concatenate(cols, 0))
    mb_flat = np.concatenate(mbcols)
    ntt = nkt[0] + nkt[1]
    mb = np.ascontiguousarray(mb_flat.reshape(ntt, 128).T)

    xT = chunked_T(xf.reshape(BT, E))
    ident = np.eye(128, dtype=np.float32).astype(_BF16)

    zero_bv = not np.any(np.asarray(bv))

    def wchunk(w, sl):
        # (E, PCN) f32 -> [128, EC, PCN] bf16 where [p, c, n] = w[c*128+p, n]
        return np.ascontiguousarray(
            np.asarray(w, dtype=np.float32)[:, sl].reshape(EC, 128, PCN)
            .transpose(1, 0, 2)).astype(_BF16)

    in_maps = []
    for c in range(N_CORES):
        sl = slice(c * PCN, (c + 1) * PCN)
        in_maps.append({
            "xT": xT, "xk": xk, "mb": mb, "ident": ident,
            "wq": wchunk(Wq, sl),
            "wk": wchunk(Wk, sl),
            "wv": wchunk(Wv, sl),
            "wo": np.ascontiguousarray(Wo[sl, :]).astype(_BF16),
            "bq": np.ascontiguousarray(bq[sl]).reshape(PCN, 1).astype(np.float32),
            "bk": np.ascontiguousarray(bk[sl]).reshape(PCN, 1).astype(np.float32),
            "bv": np.ascontiguousarray(bv[sl]).reshape(PCN, 1).astype(np.float32),
        })
    return nkt, zero_bv, in_maps


def _run(inputs, trace=False, tmpdir=None):
    from concourse.bass_utils import run_bass_kernel_spmd

    nkt, zero_bv, in_maps = _prepare(
        inputs["x"], inputs["attn_mask"], inputs["Wq"], inputs["bq"],
        inputs["Wk"], inputs["bk"], inputs["Wv"], inputs["bv"], inputs["Wo"])
    nc = _build(nkt, zero_bv)
    res = run_bass_kernel_spmd(nc, in_maps, list(range(N_CORES)),
                               trace=trace, tmpdir=tmpdir)
    yT = np.zeros((E, BT), dtype=np.float64)
    for c in range(N_CORES):
        yT += np.asarray(res.results[c]["yT"], dtype=np.float64)
    y = yT.T.astype(np.float32) + inputs["bo"].astype(np.float32)
    return y.reshape(B, T, E), res


def kernel(**inputs):
    y, _ = _run(inputs)
    return y


# revision 24
# speedup vs baseline: 2.1475x; 2.1475x over previous
"""ChaosAttention on 8 Trainium2 NeuronCores.

Sharding: tensor-parallel over heads. Each of the 8 cores owns H/8 = 2 heads
(128 of the 1024 q/k/v projection columns, 128 of the Wo rows). Every core
reads the full x (as x^T, bf16); the out-projection is row-parallel, so each
core returns a partial y^T (bf16) and the host sums the 8 partials (f64) and
adds bo.

The physics adapter produces a bias constant along the softmax axis, so it
cancels in the softmax and is skipped entirely.

Key compaction: masked keys get -inf scores in the reference, so the host
packs only the kept keys per batch (padded to a multiple of 128; pad slots
get a -1e30 exp bias), halving attention work for a ~50% mask.

v4 notes (vs the 187us baseline):
  - fp8 was tried and reverted: the attention output is an average over
    ~780 keys, so o is ~28x smaller than v and every quantization error
    keeps its RELATIVE size through the averaging - fp8 anywhere in the
    q/k/v path gives 1-3% output error vs the 2% budget.
  - v-projection computed transposed (8 wide matmuls like k) then rotated
    back with 4 PE transposes per window: a PE matmul carries ~170ns fixed
    overhead, so the old 32x 128-column form burned ~20us of PE across the
    kernel. The transposes write bf16 into a bitcast view of a PSUM f32
    tile (PSUM banks are all spoken for).
  - yT returned bf16 (host sums partials in f64): halves output DMA.
  - 1/Z via vector.reciprocal_approx_fast (~5x cheaper than reciprocal);
    custom-DVE ops and Pool broadcasts need partition-0-based operands.
  - normalize mul rides the Pool engine (all-SBUF); out-proj PSUM->SBUF
    copies split 7:1 DVE:ACT (Pool cannot touch PSUM).
  - Each HWDGE engine (SP/ACT only) owns ONE FIFO dma queue and a transfer
    carries ~0.7us fixed overhead: traffic is balanced across both queues
    in consumption order with ~512KB transfers. ACT only carries what must
    land before its exp stream starts; tiny loads ride the Pool SWDGE
    queue. y writes are quad-merged.
  - The first attention chunk is gated on [k00,q00,v00]; everything else
    interleaves into the chunk jt-loops (2 pops/jt) including the previous
    chunk's out-proj units at jt>=5, which keeps the final-chunk drain
    short.
"""

import numpy as np
import ml_dtypes

_BF16 = ml_dtypes.bfloat16

B, T, E, H, D = 2, 2048, 1024, 16, 64
BT = B * T                 # 4096 tokens
N_CORES = 8
PCN = E // N_CORES         # 128 per-core projection dims (2 heads x 64)
EC = E // 128              # 8 contraction chunks for the projections
IC = 512                   # query-chunk size
NQC = T // IC              # 4 query chunks per batch
SCALE = 1.0 / float(np.sqrt(D))

_cache = {}


def _build(nkt, zero_bv):
    """Build + schedule the per-core Bass program. nkt = packed key-tile
    counts per batch (ceil(kept/128))."""
    key = (tuple(nkt), zero_bv)
    if key in _cache:
        return _cache[key]

    from contextlib import ExitStack
    import concourse.tile as tile
    from concourse import bacc, mybir

    f32 = mybir.dt.float32
    bf16 = mybir.dt.bfloat16
    Exp = mybir.ActivationFunctionType.Exp

    base = [0, nkt[0]]              # packed key-tile offset per batch
    ntt = nkt[0] + nkt[1]           # total packed key tiles
    KP = ntt * 128                  # total packed+padded kv tokens

    nc = bacc.Bacc("TRN2", target_bir_lowering=False, debug=False,
                   num_devices=N_CORES)

    xT_d = nc.dram_tensor("xT", [128, EC, BT], bf16, kind="ExternalInput").ap()
    xk_d = nc.dram_tensor("xk", [128, EC, KP], bf16, kind="ExternalInput").ap()
    wq_d = nc.dram_tensor("wq", [128, EC, PCN], bf16, kind="ExternalInput").ap()
    wk_d = nc.dram_tensor("wk", [128, EC, PCN], bf16, kind="ExternalInput").ap()
    wv_d = nc.dram_tensor("wv", [128, EC, PCN], bf16, kind="ExternalInput").ap()
    wo_d = nc.dram_tensor("wo", [PCN, E], bf16, kind="ExternalInput").ap()
    id_d = nc.dram_tensor("ident", [128, 128], bf16, kind="ExternalInput").ap()
    bq_d = nc.dram_tensor("bq", [PCN, 1], f32, kind="ExternalInput").ap()
    bk_d = nc.dram_tensor("bk", [PCN, 1], f32, kind="ExternalInput").ap()
    bv_d = nc.dram_tensor("bv", [PCN, 1], f32, kind="ExternalInput").ap()
    mb_d = nc.dram_tensor("mb", [128, ntt], f32, kind="ExternalInput").ap()
    yT_d = nc.dram_tensor("yT", [E, BT], bf16, kind="ExternalOutput").ap()

    def kwins(b):
        """(start, size) 512-col windows over batch b's packed kv columns."""
        n = nkt[b] * 128
        return [(w * 512, min(512, n - w * 512)) for w in range((n + 511) // 512)]

    with tile.TileContext(nc) as tc, ExitStack() as ctx:
        consts = ctx.enter_context(tc.tile_pool(name="consts", bufs=1))
        pp_mm = ctx.enter_context(tc.tile_pool(name="ppmm", bufs=2, space="PSUM"))
        pp_st = ctx.enter_context(tc.tile_pool(name="ppst", bufs=2, space="PSUM"))
        pp_o = ctx.enter_context(tc.tile_pool(name="ppo", bufs=2, space="PSUM"))
        pool_pt = ctx.enter_context(tc.tile_pool(name="ptp", bufs=3))
        pool_vt = ctx.enter_context(tc.tile_pool(name="vtp", bufs=2))
        pool_oc = ctx.enter_context(tc.tile_pool(name="ocp", bufs=2))
        pool_rz = ctx.enter_context(tc.tile_pool(name="rzp", bufs=2))
        pool_rb = ctx.enter_context(tc.tile_pool(name="rbp", bufs=2))
        pool_y = ctx.enter_context(tc.tile_pool(name="yp", bufs=4))

        # ---- persistent SBUF residents ----
        xq = [consts.tile([128, EC, 512], bf16, tag=f"xq{mw}", name=f"xq{mw}")
              for mw in range(8)]
        xkw = {}
        for b in range(B):
            for wi, (w0, wsz) in enumerate(kwins(b)):
                xkw[(b, wi)] = consts.tile([128, EC, wsz], bf16,
                                           tag=f"xk{b}_{wi}", name=f"xk{b}_{wi}")

        wq_sb = consts.tile([128, EC, PCN], bf16, tag="wq")
        wk_sb = consts.tile([128, EC, PCN], bf16, tag="wk")
        wv_sb = consts.tile([128, EC, PCN], bf16, tag="wv")
        wo_sb = consts.tile([128, E], bf16, tag="wo")
        id_sb = consts.tile([128, 128], bf16, tag="ident")
        mb_sb = consts.tile([128, ntt], f32, tag="mb")
        bq_sb = consts.tile([128, 1], f32, tag="bq")
        bk_sb = consts.tile([128, 1], f32, tag="bk")
        bv_sb = consts.tile([128, 1], f32, tag="bv")

        def dma_xk(eng, b, wi, splits):
            w0, wsz = kwins(b)[wi]
            c0 = base[b] * 128 + w0
            e0 = 0
            for step in splits:
                nc_eng = eng
                nc_eng.dma_start(xkw[(b, wi)][:, e0:e0 + step, :],
                                 xk_d[:, e0:e0 + step, c0:c0 + wsz])
                e0 += step

        def dma_xq(eng, mw, splits):
            e0 = 0
            for step in splits:
                eng.dma_start(xq[mw][:, e0:e0 + step, :],
                              xT_d[:, e0:e0 + step, mw * 512:(mw + 1) * 512])
                e0 += step

        # DMA layout: ACT carries only what must land before its exp stream
        # starts; SP carries the rest of the inputs + mid-stream y writes;
        # tiny loads ride the Pool SWDGE queue.
        dma_xk(nc.sync, 0, 0, (2, 2, 4))
        for w_sb, w_d in ((wk_sb, wk_d), (wq_sb, wq_d), (wv_sb, wv_d)):
            nc.scalar.dma_start(w_sb[:], w_d[:])
        nc.gpsimd.dma_start(id_sb[:], id_d[:])
        nc.gpsimd.dma_start(mb_sb[:], mb_d[:])
        nc.gpsimd.dma_start(bq_sb[:], bq_d[:])
        nc.gpsimd.dma_start(bk_sb[:], bk_d[:])
        if not zero_bv:
            nc.gpsimd.dma_start(bv_sb[:], bv_d[:])
        dma_xq(nc.scalar, 0, (2, 2, 4))
        for wi in range(1, len(kwins(0))):
            dma_xk(nc.sync, 0, wi, (4, 4))
        dma_xq(nc.sync, 1, (4, 4))
        dma_xq(nc.sync, 2, (4, 4))
        for wi in range(len(kwins(1))):
            dma_xk(nc.sync, 1, wi, (4, 4))
        dma_xq(nc.sync, 3, (4, 4))
        nc.sync.dma_start(wo_sb[:], wo_d[:])
        for mw in range(4, 8):
            dma_xq(nc.sync, mw, (4, 4))

        # per-chunk tiles: fine-grained deps let attention start early
        qTc = {(b, icx): consts.tile([128, IC], bf16, tag=f"qT{b}_{icx}",
                                     name=f"qT{b}_{icx}")
               for b in range(B) for icx in range(NQC)}
        ATc = {(b, icx): consts.tile([128, IC], bf16, tag=f"AT{b}_{icx}",
                                     name=f"AT{b}_{icx}")
               for b in range(B) for icx in range(NQC)}
        kTc = {}
        Vpg = {}
        for b in range(B):
            for wi, (w0, wsz) in enumerate(kwins(b)):
                kTc[(b, wi)] = consts.tile([128, wsz], bf16, tag=f"kT{b}_{wi}",
                                           name=f"kT{b}_{wi}")
                Vpg[(b, wi)] = consts.tile([128, wsz // 128, 2, D + 1], bf16,
                                           tag=f"Vp{b}_{wi}", name=f"Vp{b}_{wi}")
                nc.gpsimd.memset(Vpg[(b, wi)][:, :, :, D:D + 1], 1.0)

        # ---- phase emitters ----
        def proj_q_chunk(b, icx):
            mw = (b * T + icx * IC) // 512
            ps = pp_mm.tile([128, 512], f32, tag="mm", name="psq")
            for ec in range(EC):
                nc.tensor.matmul(ps[:], lhsT=wq_sb[:, ec, :],
                                 rhs=xq[mw][:, ec, :],
                                 start=(ec == 0), stop=(ec == EC - 1))
            nc.vector.tensor_scalar_add(out=qTc[(b, icx)][:], in0=ps[:],
                                        scalar1=bq_sb[:])

        def proj_k_chunk(b, wi, wsz):
            ps = pp_mm.tile([128, 512], f32, tag="mm", name="psk")
            for ec in range(EC):
                nc.tensor.matmul(ps[:, 0:wsz], lhsT=wk_sb[:, ec, :],
                                 rhs=xkw[(b, wi)][:, ec, :],
                                 start=(ec == 0), stop=(ec == EC - 1))
            nc.vector.tensor_scalar_add(out=kTc[(b, wi)][:], in0=ps[:, 0:wsz],
                                        scalar1=bk_sb[:])

        vt_st = {}

        def proj_v_mm(b, wi, wsz):
            # v computed transposed (vdims on partitions): 8 wide matmuls
            # instead of 32 overhead-bound 128-column ones
            ps = pp_mm.tile([128, 512], f32, tag="mm", name="psv")
            for ec in range(EC):
                nc.tensor.matmul(ps[:, 0:wsz], lhsT=wv_sb[:, ec, :],
                                 rhs=xkw[(b, wi)][:, ec, :],
                                 start=(ec == 0), stop=(ec == EC - 1))
            vt = pool_vt.tile([128, 512], bf16, tag="vt", name="vt")
            if zero_bv:
                nc.vector.tensor_copy(out=vt[:, 0:wsz], in_=ps[:, 0:wsz])
            else:
                nc.vector.tensor_scalar_add(out=vt[:, 0:wsz], in0=ps[:, 0:wsz],
                                            scalar1=bv_sb[:])
            vt_st[(b, wi)] = vt

        def proj_v_tp(b, wi, wsz):
            # rotate v back to natural layout (tokens on partitions) with PE
            # transposes, staging bf16 through a bitcast view of an f32 PSUM
            # tile; then pack into Vpg next to its ones-column
            nt = wsz // 128
            vt = vt_st.pop((b, wi))
            tp = pp_mm.tile([128, 512], f32, tag="mm", name="pst")
            tpb = tp[:].bitcast(bf16)
            for q in range(nt):
                nc.tensor.transpose(tpb[:, q * 128:(q + 1) * 128],
                                    vt[:, q * 128:(q + 1) * 128], id_sb[:])
            nc.vector.tensor_copy(
                out=Vpg[(b, wi)][:, :, :, 0:D],
                in_=tpb[:, 0:wsz].rearrange("p (q h d) -> p q h d", q=nt, h=2))

        def proj_items(b):
            items = []
            wins = kwins(b)
            for wi, (w0, wsz) in enumerate(wins):
                items.append(lambda b=b, wi=wi, wsz=wsz: proj_k_chunk(b, wi, wsz))
                items.append(lambda b=b, wi=wi, wsz=wsz: proj_v_mm(b, wi, wsz))
                items.append(lambda b=b, wi=wi, wsz=wsz: proj_v_tp(b, wi, wsz))
            items.insert(1, lambda b=b: proj_q_chunk(b, 0))
            for icx in range(1, NQC):
                items.append(lambda b=b, icx=icx: proj_q_chunk(b, icx))
            return items

        def outproj_unit(b, icx, et, last=False):
            yp = pp_mm.tile([128, 512], f32, tag="mm", name="psy")
            nc.tensor.matmul(yp[:], lhsT=wo_sb[:, et * 128:(et + 1) * 128],
                             rhs=ATc[(b, icx)][:], start=True, stop=True)
            ysb = pool_y.tile([128, 4, 512], bf16, tag="y",
                              name=f"y{et // 4}") if et % 4 == 0 else \
                outproj_unit.cur
            outproj_unit.cur = ysb
            # PSUM is only reachable from PE/ACT/DVE; one copy per chunk
            # rides ACT, the rest DVE
            if et == 3:
                nc.scalar.copy(out=ysb[:, et % 4, :], in_=yp[:])
            else:
                nc.vector.tensor_copy(out=ysb[:, et % 4, :], in_=yp[:])
            if et % 4 == 3:
                i0 = b * T + icx * IC
                dst = yT_d[(et - 3) * 128:(et + 1) * 128, i0:i0 + 512]
                # the final chunk drains into the kernel tail on the ACT
                # queue, which is idle once the exps are done
                eng = nc.scalar if last else nc.sync
                eng.dma_start(dst.rearrange("(j p) n -> p j n", p=128), ysb[:])

        def outproj_units(b, icx):
            last = (b == B - 1 and icx == NQC - 1)
            return [lambda et=et: outproj_unit(b, icx, et, last)
                    for et in range(EC)]

        def attn_chunk(b, icx, light, late, heavy, norm_prev):
            o_ps = [pp_o.tile([D + 1, IC], f32, tag="o", name=f"o{h}")
                    for h in range(2)]
            for jt in range(nkt[b]):
                tg = base[b] + jt
                wi, q = jt // 4, jt % 4
                st = pp_st.tile([128, 2 * IC], f32, tag="st")
                nc.tensor.matmul(st[:, 0:IC],
                                 lhsT=kTc[(b, wi)][0:64, q * 128:(q + 1) * 128],
                                 rhs=qTc[(b, icx)][0:64, :],
                                 tile_position=(0, 0), start=True, stop=True)
                nc.tensor.matmul(st[:, IC:2 * IC],
                                 lhsT=kTc[(b, wi)][64:128, q * 128:(q + 1) * 128],
                                 rhs=qTc[(b, icx)][64:128, :],
                                 tile_position=(64, 0), start=True, stop=True)
                pt = pool_pt.tile([128, 2 * IC], bf16, tag="pt")
                nc.scalar.activation(out=pt[:], in_=st[:], func=Exp,
                                     bias=mb_sb[:, tg:tg + 1], scale=SCALE)
                for h in range(2):
                    nc.tensor.matmul(o_ps[h][:], lhsT=Vpg[(b, wi)][:, q, h, :],
                                     rhs=pt[:, h * IC:(h + 1) * IC],
                                     start=(jt == 0), stop=(jt == nkt[b] - 1))
                for _ in range(2):
                    if light:
                        light.pop(0)()
                if norm_prev and jt >= 1:
                    for _ in range(2):
                        if norm_prev:
                            norm_prev.pop(0)()
                if jt >= 5:
                    for _ in range(2):
                        if late:
                            late.pop(0)()
                if jt % 3 == 1 and heavy:
                    heavy.pop(0)()
            # free the o_ps slots fast: only the PSUM->SBUF copies happen at
            # the boundary; the reciprocal/broadcast/mul tail is deferred
            # into the next chunk, keeping ACT on back-to-back exps.
            ocs = []
            for h in range(2):
                oc = pool_oc.tile([D + 1, IC], f32, tag="oc", name=f"oc{h}")
                nc.vector.tensor_copy(out=oc[:], in_=o_ps[h][:])
                ocs.append(oc)

            st8 = {}

            def p_recip(h):
                # custom-DVE reciprocal and Pool broadcast both require
                # partition-0-based operands: gather Z to partition 0 first
                zz = pool_rz.tile([1, IC], f32, tag=f"zz{h}", name=f"zz{h}")
                nc.vector.tensor_copy(out=zz[:], in_=ocs[h][D:D + 1, :])
                rz = pool_rz.tile([1, IC], f32, tag=f"rz{h}", name=f"rz{h}")
                nc.vector.reciprocal_approx_fast(out=rz[:], in_=zz[:])
                st8[("rz", h)] = rz

            def p_bcast(h):
                rb = pool_rb.tile([D, IC], f32, tag=f"rb{h}", name=f"rb{h}")
                nc.gpsimd.partition_broadcast(rb[:], st8[("rz", h)][0:1, :])
                st8[("rb", h)] = rb

            def p_mul(h):
                # NOT on Pool: its in-order queue turns the cross-engine
                # recip->bcast->mul chain into a per-chunk latency wall
                at = ATc[(b, icx)][D * h:D * (h + 1), :]
                nc.vector.tensor_mul(out=at, in0=ocs[h][0:D, :],
                                     in1=st8[("rb", h)][:])

            return [lambda: p_recip(0), lambda: p_recip(1),
                    lambda: p_bcast(0), lambda: p_bcast(1),
                    lambda: p_mul(0), lambda: p_mul(1)]

        # ---- program ----
        # only the pieces chunk (0,0) needs before its jt loop go upfront;
        # the rest of b0's projections interleave into the chunk jt-loops
        items0 = proj_items(0)
        for it in items0[:4]:
            it()
        ready = items0[4:]
        heavy = proj_items(1)
        delay, norm2 = [], None
        for b in range(B):
            for icx in range(NQC):
                norm2 = attn_chunk(b, icx, ready, delay, heavy, norm2)
                ready.extend(delay)
                delay = outproj_units(b, icx)
        while heavy:
            heavy.pop(0)()
        for p in norm2:
            p()
        while ready:
            ready.pop(0)()
        for u in delay:
            u()

    nc.compile()
    _cache[key] = nc
    return nc


def _prepare(x, attn_mask, Wq, bq, Wk, bk, Wv, bv, Wo):
    mask = np.asarray(attn_mask).astype(bool)
    xf = np.asarray(x, dtype=np.float32).reshape(B, T, E)

    nkt = []
    cols = []       # packed kv token features, (KP, E) f32
    mbcols = []     # per packed slot: 0 keep / -1e30 pad
    for b in range(B):
        idx = np.nonzero(mask[b])[0]
        nk = len(idx)
        ntiles = max(1, (nk + 127) // 128)
        npad = ntiles * 128
        feats = np.zeros((npad, E), dtype=np.float32)
        feats[:nk] = xf[b, idx, :]
        bias = np.full(npad, -1e30, dtype=np.float32)
        bias[:nk] = 0.0
        nkt.append(ntiles)
        cols.append(feats)
        mbcols.append(bias)

    def chunked_T(a):
        # (N, E) f32 -> [128, EC, N] bf16 where [p, c, n] = a[n, c*128+p]
        return np.ascontiguousarray(
            a.T.reshape(EC, 128, -1).transpose(1, 0, 2)).astype(_BF16)

    xk = chunked_T(np.concatenate(cols, 0))
    mb_flat = np.concatenate(mbcols)
    ntt = nkt[0] + nkt[1]
    mb = np.ascontiguousarray(mb_flat.reshape(ntt, 128).T)

    xT = chunked_T(xf.reshape(BT, E))
    ident = np.eye(128, dtype=np.float32).astype(_BF16)

    zero_bv = not np.any(np.asarray(bv))

    def wchunk(w, sl):
        # (E, PCN) f32 -> [128, EC, PCN] bf16 where [p, c, n] = w[c*128+p, n]
        return np.ascontiguousarray(
            np.asarray(w, dtype=np.float32)[:, sl].reshape(EC, 128, PCN)
            .transpose(1, 0, 2)).astype(_BF16)

    in_maps = []
    for c in range(N_CORES):
        sl = slice(c * PCN, (c + 1) * PCN)
        in_maps.append({
            "xT": xT, "xk": xk, "mb": mb, "ident": ident,
            "wq": wchunk(Wq, sl),
            "wk": wchunk(Wk, sl),
            "wv": wchunk(Wv, sl),
            "wo": np.ascontiguousarray(Wo[sl, :]).astype(_BF16),
            "bq": np.ascontiguousarray(bq[sl]).reshape(PCN, 1).astype(np.float32),
            "bk": np.ascontiguousarray(bk[sl]).reshape(PCN, 1).astype(np.float32),
            "bv": np.ascontiguousarray(bv[sl]).reshape(PCN, 1).astype(np.float32),
        })
    return nkt, zero_bv, in_maps


def _run(inputs, trace=False, tmpdir=None):
    from concourse.bass_utils import run_bass_kernel_spmd

    nkt, zero_bv, in_maps = _prepare(
        inputs["x"], inputs["attn_mask"], inputs["Wq"], inputs["bq"],
        inputs["Wk"], inputs["bk"], inputs["Wv"], inputs["bv"], inputs["Wo"])
    nc = _build(nkt, zero_bv)
    res = run_bass_kernel_spmd(nc, in_maps, list(range(N_CORES)),
                               trace=trace, tmpdir=tmpdir)
    yT = np.zeros((E, BT), dtype=np.float64)
    for c in range(N_CORES):
        yT += np.asarray(res.results[c]["yT"], dtype=np.float64)
    y = yT.T.astype(np.float32) + inputs["bo"].astype(np.float32)
    return y.reshape(B, T, E), res


def kernel(**inputs):
    y, _ = _run(inputs)
    return y
